# revision 7
# baseline (speedup 1.0000x reference)
"""Trainium2 Bass kernel for nn_Decoder: edges-on-free-dim layout (V2t).

  out[e, c] = relu( sum_k (u[e] @ W[k] @ v[e]) * Wc[k, c] )

Per core (data-parallel over E): host folds the classifier into
M2[d, (c,f)], tables gathered in bf16 with dma_gather (int16 bucketed
indices, as before).  Per 256-edge block:
  - PE transposes the u/v 128-edge tiles into one PSUM tile (uT[d,e],
    vT[f,e]); ONE ScalarE copy stages both to SBUF.
  - PE: YT_c[f, e] = matmul(lhsT=M2_c[d,f], rhs=uT[d,e]) for c=0..4 into a
    6-slot PSUM tile (1KB chunks never straddle banks; double-buffered).
  - DVE: ONE scalar_tensor_tensor: prodT[f, c, e] = YT * broadcast(vT),
    bf16 out to SBUF -- the single elementwise pass (STT is 1 elem/cycle
    regardless of dtype, so fusing everything into one op minimizes DVE,
    the bottleneck engine).
  - PE: f-reduction as matmuls with prodT chunks as the STATIONARY operand
    and a ones-vector moving: r[e(partitions), c] lands in psY slot 5.
  - ACT: one fused Relu evacuates the reduce outputs to SBUF.
  - ONE DMA per 1024-edge batch writes [128, 4, 2, 5] results to HBM.
Host side un-permutes slots; relu already applied on device.
"""
import sys
import os
import math
import functools

for _p in ("/opt/trn_rl_repo", "/root/.axon_site/_ro/trn_rl_repo"):
    if os.path.isdir(_p) and _p not in sys.path:
        sys.path.insert(0, _p)

import numpy as np
import ml_dtypes

import concourse.bass as bass
import concourse.bacc as bacc
from concourse import mybir
from concourse.tile import TileContext
from concourse.bass_utils import run_bass_kernel_spmd

bf16 = ml_dtypes.bfloat16
F32 = mybir.dt.float32
BF16 = mybir.dt.bfloat16
I16 = mybir.dt.int16

N_USERS, N_ITEMS, D, E, K, C = 100000, 50000, 128, 1000000, 8, 5
NCORES = 8
EL = E // NCORES
P = 128
BATCH = 1024                  # edges per gather batch
EBLK = 256                    # edges per compute block
BPB = BATCH // EBLK           # blocks per batch = 4
CH = EBLK // P                # e-chunks per block = 2
BUCKET_ROWS = 32768
U_BUCKETS = math.ceil(N_USERS / BUCKET_ROWS)   # 4
V_BUCKETS = math.ceil(N_ITEMS / BUCKET_ROWS)   # 2
NJ = U_BUCKETS * V_BUCKETS
CF = C * D


def _default_nb():
    pu = [min(BUCKET_ROWS, N_USERS - i * BUCKET_ROWS) / N_USERS for i in range(U_BUCKETS)]
    pv = [min(BUCKET_ROWS, N_ITEMS - i * BUCKET_ROWS) / N_ITEMS for i in range(V_BUCKETS)]
    nb = []
    for iu in range(U_BUCKETS):
        for iv in range(V_BUCKETS):
            p = pu[iu] * pv[iv]
            mean = EL * p
            sig = math.sqrt(EL * p * (1 - p))
            nb.append(max(1, math.ceil((mean + 6 * sig + 1) / BATCH)))
    return tuple(nb)


DEFAULT_NB = _default_nb()


@functools.lru_cache(maxsize=4)
def _build_program(nb: tuple, reps: int = 1):
    nbtot = sum(nb)
    nblk = nbtot * BPB
    nidx_cols = nbtot * (BATCH // 16)

    nc = bacc.Bacc("TRN2", target_bir_lowering=False, debug=False,
                   num_devices=NCORES, num_swdge_queues=2)

    ut_d = nc.declare_dram_parameter("ut", [N_USERS, D], BF16, isOutput=False)
    it_d = nc.declare_dram_parameter("it", [N_ITEMS, D], BF16, isOutput=False)
    uidx_d = nc.declare_dram_parameter("uidx", [P, nidx_cols], I16, isOutput=False)
    vidx_d = nc.declare_dram_parameter("vidx", [P, nidx_cols], I16, isOutput=False)
    m2_d = nc.declare_dram_parameter("m2", [D, CF], BF16, isOutput=False)
    ones_d = nc.declare_dram_parameter("ones", [P, 1], BF16, isOutput=False)
    id_d = nc.declare_dram_parameter("ident", [P, P], BF16, isOutput=False)
    out_d = nc.declare_dram_parameter("outp", [nbtot, P, BPB, CH, C], F32,
                                      isOutput=True)

    batch_bk = []
    for jk, cnt in enumerate(nb):
        iu, iv = divmod(jk, V_BUCKETS)
        for _ in range(cnt):
            batch_bk.append((iu, iv))

    with TileContext(nc) as tc:
        with (
            tc.tile_pool(name="const", bufs=1) as const,
            tc.tile_pool(name="ug", bufs=3) as ugp,
            tc.tile_pool(name="vg", bufs=3) as vgp,
            tc.tile_pool(name="prod", bufs=3) as prp,
            tc.tile_pool(name="uT", bufs=3) as uTp,
            tc.tile_pool(name="stg", bufs=3) as stp,
            tc.tile_pool(name="psY", bufs=2, space="PSUM") as psYp,
            tc.tile_pool(name="psT", bufs=2, space="PSUM") as psTp,
        ):
            uidx_sb = const.tile([P, nidx_cols], I16)
            nc.sync.dma_start(out=uidx_sb[:], in_=uidx_d[:])
            vidx_sb = const.tile([P, nidx_cols], I16)
            nc.sync.dma_start(out=vidx_sb[:], in_=vidx_d[:])
            m2_sb = const.tile([D, CF], BF16)
            nc.sync.dma_start(out=m2_sb[:], in_=m2_d[:])
            ones_sb = const.tile([P, 1], BF16)
            nc.sync.dma_start(out=ones_sb[:], in_=ones_d[:])
            id_sb = const.tile([P, P], BF16)
            nc.sync.dma_start(out=id_sb[:], in_=id_d[:])

            tc.strict_bb_all_engine_barrier()

            # One-block software pipeline: each block's f-reduction + relu
            # (+ the batch's out-DMA) is deferred until after the NEXT
            # block's transposes/Y-matmuls are issued, so the PE never
            # stalls at the head of its in-order queue waiting for the
            # DVE multiply it feeds.
            pending = []

            def retire(p):
                psY_p, prod_p, stage_p, k_p, b_p = p
                for ch in range(CH):
                    for c in range(C):
                        nc.tensor.matmul(
                            out=psY_p[:, 5, ch, c:c + 1],
                            lhsT=prod_p[:, c, ch, :],
                            rhs=ones_sb[:],
                            start=True, stop=True,
                        )
                nc.scalar.activation(
                    out=stage_p[:, k_p, :, :], in_=psY_p[:, 5, :, 0:C],
                    func=mybir.ActivationFunctionType.Relu,
                )
                if k_p == BPB - 1:
                    nc.sync.dma_start(out=out_d[b_p], in_=stage_p[:])

            ic = BATCH // 16
            for b in [bb for _ in range(reps) for bb in range(nbtot)]:
                iu, iv = batch_bk[b]

                ubuf = ugp.tile([P, BATCH // P, D], BF16)
                nc.gpsimd.dma_gather(
                    out_ap=ubuf[:],
                    in_ap=ut_d[iu * BUCKET_ROWS:, :],
                    idxs_ap=uidx_sb[:, b * ic:(b + 1) * ic],
                    num_idxs=BATCH,
                    num_idxs_reg=BATCH,
                    elem_size=D,
                )
                vbuf = vgp.tile([P, BATCH // P, D], BF16)
                nc.gpsimd.dma_gather(
                    out_ap=vbuf[:],
                    in_ap=it_d[iv * BUCKET_ROWS:, :],
                    idxs_ap=vidx_sb[:, b * ic:(b + 1) * ic],
                    num_idxs=BATCH,
                    num_idxs_reg=BATCH,
                    elem_size=D,
                    queue_num=1,
                )

                stage = stp.tile([P, BPB, CH, C], F32)
                for k in range(BPB):
                    # on-chip PE transposes: uT/vT for this block's CH tiles
                    # (slot 0 = uT, slot 1 = vT; one PSUM bank)
                    psT = psTp.tile([P, 2, CH, P], BF16)
                    for ch in range(CH):
                        t = k * CH + ch
                        nc.tensor.transpose(out=psT[:, 0, ch, :],
                                            in_=ubuf[:, t, :], identity=id_sb[:])
                        nc.tensor.transpose(out=psT[:, 1, ch, :],
                                            in_=vbuf[:, t, :], identity=id_sb[:])
                    uvs = uTp.tile([P, 2, CH, P], BF16)
                    nc.scalar.copy(out=uvs[:], in_=psT[:])

                    # slots 0-4: YT_c (1KB chunks never straddle banks);
                    # slot 5 doubles as the reduce-output region
                    psY = psYp.tile([P, 6, CH, P], F32)
                    for c in range(C):
                        nc.tensor.matmul(
                            out=psY[:, c, :, :],
                            lhsT=m2_sb[:, c * D:(c + 1) * D],
                            rhs=uvs[:, 0, :, :],
                            start=True, stop=True,
                        )

                    if pending:
                        retire(pending.pop())

                    prod = prp.tile([P, C, CH, P], BF16)
                    nc.vector.scalar_tensor_tensor(
                        out=prod[:],
                        in0=psY[:, 0:C, :, :],
                        scalar=1.0,
                        in1=uvs[:, 1, :, :].unsqueeze(1).broadcast_to(
                            (P, C, CH, P)),
                        op0=mybir.AluOpType.mult,
                        op1=mybir.AluOpType.mult,
                    )
                    pending.append((psY, prod, stage, k, b))

            if pending:
                retire(pending.pop())

    nc.compile()
    return nc, nbtot


def _prep_core(ui, vi, nb):
    nbtot = sum(nb)
    nslots = nbtot * BATCH
    jb = (ui >> 15) * V_BUCKETS + (vi >> 15)
    u16 = np.zeros(nslots, np.int16)
    v16 = np.zeros(nslots, np.int16)
    orig = np.full(nslots, -1, np.int64)
    off = 0
    for jk in range(NJ):
        sel = np.nonzero(jb == jk)[0]
        cnt = len(sel)
        cap = nb[jk] * BATCH
        if cnt > cap:
            return None, None, None
        iu, iv = divmod(jk, V_BUCKETS)
        u16[off:off + cnt] = (ui[sel] - iu * BUCKET_ROWS).astype(np.int16)
        v16[off:off + cnt] = (vi[sel] - iv * BUCKET_ROWS).astype(np.int16)
        orig[off:off + cnt] = sel
        off += cap
    return u16, v16, orig


def _wrap_idx(x16, nbtot):
    w = x16.reshape(nbtot, BATCH // 16, 16).transpose(2, 0, 1).reshape(16, -1)
    return np.ascontiguousarray(np.tile(w, (8, 1)))


def _prepare(user_inputs, item_inputs, user_indices, item_indices,
             weight, weight_classifier):
    user_inputs = np.asarray(user_inputs)
    item_inputs = np.asarray(item_inputs)
    ui_all = np.asarray(user_indices).astype(np.int64)
    vi_all = np.asarray(item_indices).astype(np.int64)
    weight = np.asarray(weight, dtype=np.float32)
    wc = np.asarray(weight_classifier, dtype=np.float32)

    m2 = np.einsum("kdf,kc->cdf", weight, wc).transpose(1, 0, 2).reshape(D, CF)
    m2 = np.ascontiguousarray(m2).astype(bf16)
    ut_bf = np.ascontiguousarray(user_inputs.astype(bf16))
    it_bf = np.ascontiguousarray(item_inputs.astype(bf16))
    ones = np.ones((P, 1), dtype=bf16)
    ident = np.eye(P, dtype=np.float32).astype(bf16)

    nb = DEFAULT_NB
    while True:
        preps = []
        ok = True
        for c in range(NCORES):
            seg = slice(c * EL, (c + 1) * EL)
            u16, v16, orig = _prep_core(ui_all[seg], vi_all[seg], nb)
            if u16 is None:
                ok = False
                break
            preps.append((u16, v16, orig))
        if ok:
            break
        counts = np.zeros(NJ, np.int64)
        for c in range(NCORES):
            seg = slice(c * EL, (c + 1) * EL)
            jb = (ui_all[seg] >> 15) * V_BUCKETS + (vi_all[seg] >> 15)
            counts = np.maximum(counts, np.bincount(jb, minlength=NJ))
        nb = tuple(int(math.ceil((cn + 1) / BATCH)) for cn in counts)

    nc, nbtot = _build_program(nb)

    in_maps = []
    for c in range(NCORES):
        u16, v16, orig = preps[c]
        in_maps.append({
            "ut": ut_bf,
            "it": it_bf,
            "uidx": _wrap_idx(u16, nbtot),
            "vidx": _wrap_idx(v16, nbtot),
            "m2": m2,
            "ones": ones,
            "ident": ident,
        })

    return nc, nbtot, in_maps, preps, nb


def _postprocess(results, nbtot, preps):
    out = np.empty((E, C), np.float32)
    for c in range(NCORES):
        o = results[c]["outp"]              # [nbtot, P, BPB, CH, C]
        slotted = o.transpose(0, 2, 3, 1, 4).reshape(-1, C)
        _, _, orig = preps[c]
        mask = orig >= 0
        out[c * EL + orig[mask]] = slotted[mask]
    return out


def kernel(user_inputs, item_inputs, user_indices, item_indices,
           weight, weight_classifier):
    nc, nbtot, in_maps, preps, nb = _prepare(
        user_inputs, item_inputs, user_indices, item_indices,
        weight, weight_classifier)
    results = run_bass_kernel_spmd(nc, in_maps, list(range(NCORES))).results
    return _postprocess(results, nbtot, preps)


# revision 10
# speedup vs baseline: 1.0096x; 1.0096x over previous
"""Trainium2 Bass kernel for nn_Decoder: edges-on-free-dim layout (V2t).

  out[e, c] = relu( sum_k (u[e] @ W[k] @ v[e]) * Wc[k, c] )

Per core (data-parallel over E): host folds the classifier into
M2[d, (c,f)], tables gathered in bf16 with dma_gather (int16 bucketed
indices, as before).  Per 256-edge block:
  - PE transposes the u/v 128-edge tiles into one PSUM tile (uT[d,e],
    vT[f,e]); ONE ScalarE copy stages both to SBUF.
  - PE: YT_c[f, e] = matmul(lhsT=M2_c[d,f], rhs=uT[d,e]) for c=0..4 into a
    6-slot PSUM tile (1KB chunks never straddle banks; double-buffered).
  - DVE: ONE scalar_tensor_tensor: prodT[f, c, e] = YT * broadcast(vT),
    bf16 out to SBUF -- the single elementwise pass (STT is 1 elem/cycle
    regardless of dtype, so fusing everything into one op minimizes DVE,
    the bottleneck engine).
  - PE: f-reduction as matmuls with prodT chunks as the STATIONARY operand
    and a ones-vector moving: r[e(partitions), c] lands in psY slot 5.
  - ACT: one fused Relu evacuates the reduce outputs to SBUF.
  - ONE DMA per 1024-edge batch writes [128, 4, 2, 5] results to HBM.
Host side un-permutes slots; relu already applied on device.
"""
import sys
import os
import math
import functools

for _p in ("/opt/trn_rl_repo", "/root/.axon_site/_ro/trn_rl_repo"):
    if os.path.isdir(_p) and _p not in sys.path:
        sys.path.insert(0, _p)

import numpy as np
import ml_dtypes

import concourse.bass as bass
import concourse.bacc as bacc
from concourse import mybir
from concourse.tile import TileContext
from concourse.bass_utils import run_bass_kernel_spmd

bf16 = ml_dtypes.bfloat16
F32 = mybir.dt.float32
BF16 = mybir.dt.bfloat16
I16 = mybir.dt.int16

N_USERS, N_ITEMS, D, E, K, C = 100000, 50000, 128, 1000000, 8, 5
NCORES = 8
EL = E // NCORES
P = 128
BATCH = 1024                  # edges per gather batch
EBLK = 256                    # edges per compute block
BPB = BATCH // EBLK           # blocks per batch = 4
CH = EBLK // P                # e-chunks per block = 2
BUCKET_ROWS = 32768
U_BUCKETS = math.ceil(N_USERS / BUCKET_ROWS)   # 4
V_BUCKETS = math.ceil(N_ITEMS / BUCKET_ROWS)   # 2
NJ = U_BUCKETS * V_BUCKETS
CF = C * D


def _default_nb():
    pu = [min(BUCKET_ROWS, N_USERS - i * BUCKET_ROWS) / N_USERS for i in range(U_BUCKETS)]
    pv = [min(BUCKET_ROWS, N_ITEMS - i * BUCKET_ROWS) / N_ITEMS for i in range(V_BUCKETS)]
    nb = []
    for iu in range(U_BUCKETS):
        for iv in range(V_BUCKETS):
            p = pu[iu] * pv[iv]
            mean = EL * p
            sig = math.sqrt(EL * p * (1 - p))
            nb.append(max(1, math.ceil((mean + 6 * sig + 1) / BATCH)))
    return tuple(nb)


DEFAULT_NB = _default_nb()


@functools.lru_cache(maxsize=4)
def _build_program(nb: tuple, reps: int = 1):
    nbtot = sum(nb)
    nblk = nbtot * BPB
    nidx_cols = nbtot * (BATCH // 16)

    nc = bacc.Bacc("TRN2", target_bir_lowering=False, debug=False,
                   num_devices=NCORES, num_swdge_queues=2)

    ut_d = nc.declare_dram_parameter("ut", [N_USERS, D], BF16, isOutput=False)
    it_d = nc.declare_dram_parameter("it", [N_ITEMS, D], BF16, isOutput=False)
    uidx_d = nc.declare_dram_parameter("uidx", [P, nidx_cols], I16, isOutput=False)
    vidx_d = nc.declare_dram_parameter("vidx", [P, nidx_cols], I16, isOutput=False)
    m2_d = nc.declare_dram_parameter("m2", [D, CF], BF16, isOutput=False)
    ones_d = nc.declare_dram_parameter("ones", [P, 1], BF16, isOutput=False)
    id_d = nc.declare_dram_parameter("ident", [P, P], BF16, isOutput=False)
    out_d = nc.declare_dram_parameter("outp", [nbtot, P, BPB, CH, C], F32,
                                      isOutput=True)

    batch_bk = []
    for jk, cnt in enumerate(nb):
        iu, iv = divmod(jk, V_BUCKETS)
        for _ in range(cnt):
            batch_bk.append((iu, iv))

    with TileContext(nc) as tc:
        with (
            tc.tile_pool(name="const", bufs=1) as const,
            tc.tile_pool(name="ug", bufs=3) as ugp,
            tc.tile_pool(name="vg", bufs=3) as vgp,
            tc.tile_pool(name="prod", bufs=3) as prp,
            tc.tile_pool(name="uT", bufs=3) as uTp,
            tc.tile_pool(name="vT", bufs=3) as vTp,
            tc.tile_pool(name="stg", bufs=3) as stp,
            tc.tile_pool(name="psY", bufs=2, space="PSUM") as psYp,
            tc.tile_pool(name="psT", bufs=2, space="PSUM") as psTp,
        ):
            uidx_sb = const.tile([P, nidx_cols], I16)
            nc.sync.dma_start(out=uidx_sb[:], in_=uidx_d[:])
            vidx_sb = const.tile([P, nidx_cols], I16)
            nc.sync.dma_start(out=vidx_sb[:], in_=vidx_d[:])
            m2_sb = const.tile([D, CF], BF16)
            nc.sync.dma_start(out=m2_sb[:], in_=m2_d[:])
            ones_sb = const.tile([P, 1], BF16)
            nc.sync.dma_start(out=ones_sb[:], in_=ones_d[:])
            id_sb = const.tile([P, P], BF16)
            nc.sync.dma_start(out=id_sb[:], in_=id_d[:])

            tc.strict_bb_all_engine_barrier()

            # One-block software pipeline: each block's f-reduction + relu
            # (+ the batch's out-DMA) is deferred until after the NEXT
            # block's transposes/Y-matmuls are issued, so the PE never
            # stalls at the head of its in-order queue waiting for the
            # DVE multiply it feeds.
            pending = []

            def retire(p):
                psY_p, prod_p, stage_p, k_p, b_p = p
                for ch in range(CH):
                    for c in range(C):
                        nc.tensor.matmul(
                            out=psY_p[:, 5, ch, c:c + 1],
                            lhsT=prod_p[:, c, ch, :],
                            rhs=ones_sb[:],
                            start=True, stop=True,
                        )
                nc.scalar.activation(
                    out=stage_p[:, k_p, :, :], in_=psY_p[:, 5, :, 0:C],
                    func=mybir.ActivationFunctionType.Relu,
                )
                if k_p == BPB - 1:
                    nc.sync.dma_start(out=out_d[b_p], in_=stage_p[:])

            ic = BATCH // 16
            for b in [bb for _ in range(reps) for bb in range(nbtot)]:
                iu, iv = batch_bk[b]

                ubuf = ugp.tile([P, BATCH // P, D], BF16)
                nc.gpsimd.dma_gather(
                    out_ap=ubuf[:],
                    in_ap=ut_d[iu * BUCKET_ROWS:, :],
                    idxs_ap=uidx_sb[:, b * ic:(b + 1) * ic],
                    num_idxs=BATCH,
                    num_idxs_reg=BATCH,
                    elem_size=D,
                )
                vbuf = vgp.tile([P, BATCH // P, D], BF16)
                nc.gpsimd.dma_gather(
                    out_ap=vbuf[:],
                    in_ap=it_d[iv * BUCKET_ROWS:, :],
                    idxs_ap=vidx_sb[:, b * ic:(b + 1) * ic],
                    num_idxs=BATCH,
                    num_idxs_reg=BATCH,
                    elem_size=D,
                    queue_num=1,
                )

                stage = stp.tile([P, BPB, CH, C], F32)
                for k in range(BPB):
                    # on-chip PE transposes: uT/vT for this block's CH tiles
                    # (slot 0 = uT, slot 1 = vT; one PSUM bank)
                    psT = psTp.tile([P, 2, CH, P], BF16)
                    for ch in range(CH):
                        t = k * CH + ch
                        nc.tensor.transpose(out=psT[:, 0, ch, :],
                                            in_=ubuf[:, t, :], identity=id_sb[:])
                        nc.tensor.transpose(out=psT[:, 1, ch, :],
                                            in_=vbuf[:, t, :], identity=id_sb[:])
                    # split staging: Y only waits on the uT copy; the vT
                    # copy overlaps with the Y matmuls
                    uTs = uTp.tile([P, CH, P], BF16)
                    nc.scalar.copy(out=uTs[:], in_=psT[:, 0, :, :])
                    vTs = vTp.tile([P, CH, P], BF16)
                    nc.scalar.copy(out=vTs[:], in_=psT[:, 1, :, :])

                    # slots 0-4: YT_c (1KB chunks never straddle banks);
                    # slot 5 doubles as the reduce-output region
                    psY = psYp.tile([P, 6, CH, P], F32)
                    for c in range(C):
                        nc.tensor.matmul(
                            out=psY[:, c, :, :],
                            lhsT=m2_sb[:, c * D:(c + 1) * D],
                            rhs=uTs[:],
                            start=True, stop=True,
                        )

                    if pending:
                        retire(pending.pop())

                    prod = prp.tile([P, C, CH, P], BF16)
                    nc.vector.scalar_tensor_tensor(
                        out=prod[:],
                        in0=psY[:, 0:C, :, :],
                        scalar=1.0,
                        in1=vTs[:].unsqueeze(1).broadcast_to(
                            (P, C, CH, P)),
                        op0=mybir.AluOpType.mult,
                        op1=mybir.AluOpType.mult,
                    )
                    pending.append((psY, prod, stage, k, b))

            if pending:
                retire(pending.pop())

    nc.compile()
    return nc, nbtot


def _prep_core(ui, vi, nb):
    nbtot = sum(nb)
    nslots = nbtot * BATCH
    jb = (ui >> 15) * V_BUCKETS + (vi >> 15)
    u16 = np.zeros(nslots, np.int16)
    v16 = np.zeros(nslots, np.int16)
    orig = np.full(nslots, -1, np.int64)
    off = 0
    for jk in range(NJ):
        sel = np.nonzero(jb == jk)[0]
        cnt = len(sel)
        cap = nb[jk] * BATCH
        if cnt > cap:
            return None, None, None
        iu, iv = divmod(jk, V_BUCKETS)
        u16[off:off + cnt] = (ui[sel] - iu * BUCKET_ROWS).astype(np.int16)
        v16[off:off + cnt] = (vi[sel] - iv * BUCKET_ROWS).astype(np.int16)
        orig[off:off + cnt] = sel
        off += cap
    return u16, v16, orig


def _wrap_idx(x16, nbtot):
    w = x16.reshape(nbtot, BATCH // 16, 16).transpose(2, 0, 1).reshape(16, -1)
    return np.ascontiguousarray(np.tile(w, (8, 1)))


def _prepare(user_inputs, item_inputs, user_indices, item_indices,
             weight, weight_classifier):
    user_inputs = np.asarray(user_inputs)
    item_inputs = np.asarray(item_inputs)
    ui_all = np.asarray(user_indices).astype(np.int64)
    vi_all = np.asarray(item_indices).astype(np.int64)
    weight = np.asarray(weight, dtype=np.float32)
    wc = np.asarray(weight_classifier, dtype=np.float32)

    m2 = np.einsum("kdf,kc->cdf", weight, wc).transpose(1, 0, 2).reshape(D, CF)
    m2 = np.ascontiguousarray(m2).astype(bf16)
    ut_bf = np.ascontiguousarray(user_inputs.astype(bf16))
    it_bf = np.ascontiguousarray(item_inputs.astype(bf16))
    ones = np.ones((P, 1), dtype=bf16)
    ident = np.eye(P, dtype=np.float32).astype(bf16)

    nb = DEFAULT_NB
    while True:
        preps = []
        ok = True
        for c in range(NCORES):
            seg = slice(c * EL, (c + 1) * EL)
            u16, v16, orig = _prep_core(ui_all[seg], vi_all[seg], nb)
            if u16 is None:
                ok = False
                break
            preps.append((u16, v16, orig))
        if ok:
            break
        counts = np.zeros(NJ, np.int64)
        for c in range(NCORES):
            seg = slice(c * EL, (c + 1) * EL)
            jb = (ui_all[seg] >> 15) * V_BUCKETS + (vi_all[seg] >> 15)
            counts = np.maximum(counts, np.bincount(jb, minlength=NJ))
        nb = tuple(int(math.ceil((cn + 1) / BATCH)) for cn in counts)

    nc, nbtot = _build_program(nb)

    in_maps = []
    for c in range(NCORES):
        u16, v16, orig = preps[c]
        in_maps.append({
            "ut": ut_bf,
            "it": it_bf,
            "uidx": _wrap_idx(u16, nbtot),
            "vidx": _wrap_idx(v16, nbtot),
            "m2": m2,
            "ones": ones,
            "ident": ident,
        })

    return nc, nbtot, in_maps, preps, nb


def _postprocess(results, nbtot, preps):
    out = np.empty((E, C), np.float32)
    for c in range(NCORES):
        o = results[c]["outp"]              # [nbtot, P, BPB, CH, C]
        slotted = o.transpose(0, 2, 3, 1, 4).reshape(-1, C)
        _, _, orig = preps[c]
        mask = orig >= 0
        out[c * EL + orig[mask]] = slotted[mask]
    return out


def kernel(user_inputs, item_inputs, user_indices, item_indices,
           weight, weight_classifier):
    nc, nbtot, in_maps, preps, nb = _prepare(
        user_inputs, item_inputs, user_indices, item_indices,
        weight, weight_classifier)
    results = run_bass_kernel_spmd(nc, in_maps, list(range(NCORES))).results
    return _postprocess(results, nbtot, preps)
